# revision 1
# baseline (speedup 1.0000x reference)
"""Supervised-contrastive point-cloud loss on Trainium2 (8 NeuronCores).

Inputs (full): features [8, 128, 4096] f32, labels_all [8, 4096] int32.
Sharding: data-parallel over the batch dim — core b computes the full
4096x4096 per-cloud loss for cloud b; the host averages the 8 scalars.

Per-core algorithm (N=4096 points, C=128 channels, 16 classes):
  v = normalize(f columns)                       (cos matrix prep)
  G = v^T v  in 128-row blocks (bf16 matmuls)
  kill diagonal: G_ii -= 1e5 in PSUM  =>  exp(10*G_ii) underflows to 0
  dp = exp(10 * G)  on the scalar engine (bf16 out)
  CS[c, j] = sum_i onehot[c, i] * dp[i, j]  via a second matmul whose
      stationary operand is the one-hot label matrix (+ a ones row at
      partition 32).  dp is symmetric, so CS[label_j, j] = positives_j
      and CS[32, j] = positives_j + negatives_j.
CS is DMA'd to DRAM; the host does the O(N) tail:
  dev_j = ln(total_j) - ln(pos_j);  loss = mean_b mean_j dev_j
"""

import contextlib
import sys

for _p in ("/opt/trn_rl_repo",):
    if _p not in sys.path:
        sys.path.append(_p)

import numpy as np
import ml_dtypes

import concourse.bass as bass  # noqa: F401  (engine types referenced via nc)
import concourse.bacc as bacc
import concourse.tile as tile
from concourse import mybir
from concourse.bass_utils import run_bass_kernel_spmd

F32 = mybir.dt.float32
BF16 = mybir.dt.bfloat16
AF = mybir.ActivationFunctionType
ALU = mybir.AluOpType

B, C, N = 8, 128, 4096
NCLS = 16
TEMP_INV = 10.0  # 1 / 0.1
NBLK = N // 128          # 32 row blocks
CHUNK = 1024             # outer column chunk
NCHUNK = N // CHUNK      # 4
BIGDIAG = 1.0e5          # G_ii - 1e5, then exp(10*(..)) == 0.0
NROW = 33                # 16 one-hot rows + 16 pad + totals row at partition 32


def build_program():
    nc = bacc.Bacc("TRN2", target_bir_lowering=False, debug=False, num_devices=B)

    f_d = nc.dram_tensor("f", [C, N], F32, kind="ExternalInput").ap()
    y17_d = nc.dram_tensor("y17", [C, NBLK * NROW], BF16, kind="ExternalInput").ap()
    bigeye_d = nc.dram_tensor("bigeye", [128, 128], F32, kind="ExternalInput").ap()
    onescol_d = nc.dram_tensor("onescol", [128, 1], F32, kind="ExternalInput").ap()
    onesrow_d = nc.dram_tensor("onesrow", [1, 128], F32, kind="ExternalInput").ap()
    cs_d = nc.dram_tensor("csout", [NROW, N], F32, kind="ExternalOutput").ap()

    with tile.TileContext(nc) as tc, contextlib.ExitStack() as _stack:
        with (
            tc.tile_pool(name="const", bufs=1) as constp,
            tc.tile_pool(name="big", bufs=1) as bigp,
            tc.tile_pool(name="dp", bufs=6) as dpp,
        ):
            # ---- constants / inputs to SBUF ----
            y17_sb = constp.tile([C, NBLK * NROW], BF16)
            nc.sync.dma_start(y17_sb[:], y17_d[:])
            bigeye_sb = constp.tile([128, 128], F32)
            nc.sync.dma_start(bigeye_sb[:], bigeye_d[:])
            onescol_sb = constp.tile([128, 1], F32)
            nc.sync.dma_start(onescol_sb[:], onescol_d[:])
            onesrow_sb = constp.tile([1, 128], F32)
            nc.sync.dma_start(onesrow_sb[:], onesrow_d[:])
            tiny_sb = constp.tile([1, 1], F32)
            nc.gpsimd.memset(tiny_sb[:], 1e-30)

            f_sb = bigp.tile([C, N], F32)
            fsq = bigp.tile([C, N], F32)
            s2row = bigp.tile([1, N], F32)
            v_sb = bigp.tile([C, N], BF16)

            # ---- norms, pipelined per 1024 columns ----
            with tc.tile_pool(name="pmisc", bufs=2, space="PSUM") as pmiscp:
                lnrow = bigp.tile([1, N], F32)
                rnrow = bigp.tile([1, N], F32)
                # phase-ordered emission: each engine's program order matches
                # dependency order globally, so groups pipeline instead of
                # serializing through a per-group chain.
                for k in range(N // 1024):
                    sl = slice(k * 1024, (k + 1) * 1024)
                    nc.sync.dma_start(f_sb[:, sl], f_d[:, sl])
                for k in range(N // 1024):
                    sl = slice(k * 1024, (k + 1) * 1024)
                    nc.vector.tensor_tensor(
                        fsq[:, sl], f_sb[:, sl], f_sb[:, sl], op=ALU.mult
                    )
                s2_list = []
                for k in range(N // 512):
                    s2_ps = pmiscp.tile([1, 512], F32, tag="pm")
                    nc.tensor.matmul(
                        s2_ps[:], onescol_sb[:], fsq[:, k * 512 : (k + 1) * 512],
                        start=True, stop=True,
                    )
                    s2_list.append(s2_ps)
                # rn = 1/sqrt(s2) = exp(-0.5*ln(s2 + tiny)) — one ACT table set
                for k, s2_ps in enumerate(s2_list):
                    nc.scalar.activation(
                        lnrow[0:1, k * 512 : (k + 1) * 512], s2_ps[:], AF.Ln,
                        bias=tiny_sb[0:1, 0:1],
                    )
                for h in range(2):
                    sl = slice(h * 2048, (h + 1) * 2048)
                    nc.scalar.activation(rnrow[0:1, sl], lnrow[0:1, sl], AF.Exp, scale=-0.5)
                # v = f * rn (broadcast rn over partitions via K=1 matmul)
                bc_list = []
                for k in range(N // 512):
                    bc_ps = pmiscp.tile([128, 512], F32, tag="pm2")
                    nc.tensor.matmul(
                        bc_ps[:], onesrow_sb[:], rnrow[0:1, k * 512 : (k + 1) * 512],
                        start=True, stop=True,
                    )
                    bc_list.append(bc_ps)
                for k, bc_ps in enumerate(bc_list):
                    nc.vector.tensor_tensor(
                        v_sb[:, k * 512 : (k + 1) * 512],
                        f_sb[:, k * 512 : (k + 1) * 512],
                        bc_ps[:], op=ALU.mult,
                    )

            # ---- main loop: G -> exp -> class-sum matmul ----
            pgp = _stack.enter_context(tc.tile_pool(name="pg", bufs=3, space="PSUM"))
            pcsp = _stack.enter_context(tc.tile_pool(name="pcs", bufs=1, space="PSUM"))
            for c in range(NCHUNK):
                c0 = c * CHUNK
                cs = pcsp.tile([NROW, CHUNK], F32)

                def emit_cs(m, dp):
                    lhs = y17_sb[:, m * NROW : (m + 1) * NROW]
                    for h in range(CHUNK // 512):
                        nc.tensor.matmul(
                            cs[:, h * 512 : (h + 1) * 512],
                            lhs,
                            dp[:, h * 512 : (h + 1) * 512],
                            start=(m == 0),
                            stop=(m == NBLK - 1),
                        )

                pending = []
                for m in range(NBLK):
                    g = pgp.tile([128, CHUNK], F32)
                    lhs = v_sb[:, m * 128 : (m + 1) * 128]
                    for h in range(CHUNK // 512):
                        nc.tensor.matmul(
                            g[:, h * 512 : (h + 1) * 512],
                            lhs,
                            v_sb[:, c0 + h * 512 : c0 + (h + 1) * 512],
                            start=True, stop=True,
                        )
                    off = m * 128 - c0
                    if 0 <= off < CHUNK:
                        nc.vector.tensor_tensor(
                            g[:, off : off + 128], g[:, off : off + 128],
                            bigeye_sb[:], op=ALU.subtract,
                        )
                    dp = dpp.tile([128, CHUNK], BF16)
                    nc.scalar.activation(dp[:], g[:], AF.Exp, scale=TEMP_INV)
                    pending.append((m, dp))
                    if len(pending) > 2:
                        emit_cs(*pending.pop(0))
                for p in pending:
                    emit_cs(*p)

                cs_sb = dpp.tile([NROW, CHUNK], F32, tag="cssb")
                nc.vector.tensor_copy(cs_sb[:], cs[:])
                nc.sync.dma_start(cs_d[:, c0 : c0 + CHUNK], cs_sb[:])

    nc.compile()
    return nc


_NC = None


def _get_program():
    global _NC
    if _NC is None:
        _NC = build_program()
    return _NC


def make_in_maps(features, labels_all):
    feats = np.ascontiguousarray(np.asarray(features, dtype=np.float32))
    labels = np.asarray(labels_all, dtype=np.int32)
    onehot = (labels[:, :, None] == np.arange(NCLS)[None, None, :])  # [B, N, 16]
    y17 = np.zeros((B, N, NROW), dtype=ml_dtypes.bfloat16)
    y17[:, :, :NCLS] = onehot
    y17[:, :, NROW - 1] = 1.0
    # [N, NROW] -> [128, NBLK*NROW] so the per-block lhsT slices are contiguous
    y17p = np.ascontiguousarray(
        y17.reshape(B, NBLK, 128, NROW).transpose(0, 2, 1, 3).reshape(B, 128, NBLK * NROW)
    )
    bigeye = np.eye(128, dtype=np.float32) * BIGDIAG
    onescol = np.ones((128, 1), np.float32)
    onesrow = np.ones((1, 128), np.float32)
    return [
        {
            "f": feats[b],
            "y17": y17p[b],
            "bigeye": bigeye,
            "onescol": onescol,
            "onesrow": onesrow,
        }
        for b in range(B)
    ]


def finish_on_host(cs_all, labels_all):
    """cs_all: list of [NROW, N] per cloud. Gather + log + mean (tiny, O(N))."""
    labels = np.asarray(labels_all, dtype=np.int64)
    losses = []
    for b in range(B):
        cs = np.asarray(cs_all[b], dtype=np.float64)
        pos = cs[labels[b], np.arange(N)]
        tot = cs[NROW - 1]
        dev = np.log(tot) - np.log(pos)
        losses.append(dev.mean())
    return np.asarray(np.float32(np.mean(losses)))


def run(features, labels_all, **spmd_kwargs):
    nc = _get_program()
    in_maps = make_in_maps(features, labels_all)
    res = run_bass_kernel_spmd(nc, in_maps, list(range(B)), **spmd_kwargs)
    out = finish_on_host([res.results[b]["csout"] for b in range(B)], labels_all)
    return out, res


def kernel(features, labels_all):
    out, _ = run(features, labels_all)
    return out

